# revision 1
# baseline (speedup 1.0000x reference)
"""Trainium2 Bass kernel for a fused LSTM cell.

Problem: B=8192, I=H=1024.
  gates = [x, h_prev] @ [W_f|W_i|W_o|W_C] + b      # [B, 4H]
  C_t = sigmoid(f)*C_prev + sigmoid(i)*tanh(c)
  h_t = sigmoid(o)*tanh(C_t)

Sharding: data-parallel over batch across 8 NeuronCores (1024 rows each),
weights replicated. No collectives needed.

Per-core device program (fp32r matmuls — full PE rate, ~1e-4 rel err):
  - combined^T (K=2048 x M=1024) resident in SBUF, K on partitions.
  - W streamed as [128, 16, 128] tiles (K-chunk x N-chunk), hidden dim on
    PSUM partitions so the per-gate bias rides the ScalarE activation's
    per-partition bias operand: gate = act(psum + b).
  - Loop q (8 H-chunks of 128) x m (2 batch-chunks of 512): 4 gates x 16
    K-chunks of matmuls into 4 PSUM banks, ScalarE sigmoid/tanh eviction,
    VectorE elementwise for C_t / h_t, DMA out in [H, B] layout
    (untransposed on host).

All host-side layout shuffles (transpose/concat/reorder) are numpy copies
outside the measured device execution.
"""

import numpy as np

import concourse.bass as bass
import concourse.mybir as mybir
import concourse.tile as tile
from concourse import bacc
from concourse.bass_utils import run_bass_kernel_spmd

N_CORES = 8
B, I, H = 8192, 1024, 1024
K = I + H                      # 2048 contraction dim
BL = B // N_CORES              # 1024 batch rows per core
KC = K // 128                  # 16 K-chunks
QC = H // 128                  # 8 hidden chunks of 128
MC = 2                         # batch chunks of 512 per core
MT = BL // MC                  # 512
NCHUNKS = 4 * QC               # 32 (q-major, gate-minor) N-chunks of 128

_DT_MM = mybir.dt.float32r     # matmul operand dtype (fp32 bits, fast path)


def set_mm_dtype(name):
    """Switch matmul operand dtype ('fp32r' | 'bf16' | 'fp32'). Test-only."""
    global _DT_MM, _NP_MM
    import ml_dtypes
    _DT_MM = {"fp32r": mybir.dt.float32r, "bf16": mybir.dt.bfloat16,
              "fp32": mybir.dt.float32}[name]
    _NP_MM = ml_dtypes.bfloat16 if name == "bf16" else np.float32
    _NC_CACHE.clear()


_NP_MM = np.float32

# chain order within a group: f, i, C~ (tanh), o — o last so the final
# epilogue's critical path after the last matmul is just sigmoid(o)*tanh(C_t)
GATE_ORDER = (0, 1, 3, 2)

_SIG = mybir.ActivationFunctionType.Sigmoid
_TANH = mybir.ActivationFunctionType.Tanh


def build_program(repeats: int = 1):
    """Build the per-core Bass program. `repeats` unrolls the whole body
    (same data) for slope-based HW timing in test harnesses."""
    nc = bacc.Bacc("TRN2", target_bir_lowering=False, debug=False)

    # Host-prepped layouts (see prep_inputs):
    #   comb: [128, KC, BL]   combined^T, partition-major contiguous
    #   w:    [NCHUNKS, 128, KC, 128]  W tiles, partition-major contiguous
    #   bt:   [128, NCHUNKS]  bias chunks
    #   cp:   [128, QC, BL]   C_prev^T
    comb_d = nc.dram_tensor("comb", [128, KC, BL], _DT_MM, kind="ExternalInput")
    w_d = nc.dram_tensor("w", [NCHUNKS, 128, KC, 128], _DT_MM, kind="ExternalInput")
    bt_d = nc.dram_tensor("bt", [128, NCHUNKS], mybir.dt.float32, kind="ExternalInput")
    cp_d = nc.dram_tensor("cp", [128, QC, BL], mybir.dt.float32, kind="ExternalInput")
    ht_d = nc.dram_tensor("ht", [QC, 128, BL], mybir.dt.float32, kind="ExternalOutput")
    ct_d = nc.dram_tensor("ct", [QC, 128, BL], mybir.dt.float32, kind="ExternalOutput")

    with tile.TileContext(nc) as tc:
        with (
            tc.tile_pool(name="res", bufs=1) as res,
            tc.tile_pool(name="wp", bufs=16) as wp,
            tc.tile_pool(name="cpp", bufs=4) as cpp,
            tc.tile_pool(name="gp", bufs=2) as gp,
            tc.tile_pool(name="ep", bufs=2) as ep,
            tc.tile_pool(name="psum", bufs=2, space="PSUM") as pp,
        ):
            # q0's W tiles split into k-quarters so the first accumulation
            # chain starts after ~256KB instead of 1MB; combined^T split per
            # (K-chunk, m-half) for the same reason. GATE_ORDER puts the o
            # gate last so the final epilogue only waits on one activation.
            KSUB = 4
            # DMA emission ordered by first-use time: gate-0 W quarters, then
            # the m=0 combined chunks its chain consumes, then the remaining
            # gates' W, then the m=1 chunks.
            wts0 = [[] for _ in range(4)]
            cts = [[None] * MC for _ in range(KC)]

            def _load_wq0(g):
                for kq in range(KC // KSUB):
                    wt = wp.tile([128, KSUB, 128], _DT_MM, tag="wq0", name=f"wt0_{g}_{kq}")
                    nc.sync.dma_start(
                        out=wt[:], in_=w_d.ap()[GATE_ORDER[g], :, kq * KSUB : (kq + 1) * KSUB, :]
                    )
                    wts0[g].append(wt)

            def _load_ct(k, m):
                ctk = res.tile([128, MT], _DT_MM, tag=f"ct{k}_{m}", name=f"ct{k}_{m}")
                nc.sync.dma_start(
                    out=ctk[:], in_=comb_d.ap()[:, k, m * MT : (m + 1) * MT]
                )
                cts[k][m] = ctk

            _load_wq0(0)
            for k in range(KC):
                _load_ct(k, 0)
            _load_wq0(1)
            bt_sb = res.tile([128, NCHUNKS], mybir.dt.float32)
            nc.sync.dma_start(out=bt_sb[:], in_=bt_d.ap())
            _load_wq0(2)
            _load_wq0(3)
            for k in range(KC):
                for m in range(1, MC):
                    _load_ct(k, m)

            for _ in range(repeats):
                for q in range(QC):
                    if q == 0 and wts0 is not None:
                        wts = wts0
                        wts0 = None
                        ksub = KSUB
                    else:
                        # halve W tiles so each chain starts after 512KB, not
                        # 1MB — closes PE wait-for-W gaps in the early phase
                        ksub = KC // 2
                        wts = []
                        for g in range(4):
                            c = q * 4 + GATE_ORDER[g]
                            halves = []
                            for h2 in range(2):
                                wt = wp.tile([128, ksub, 128], _DT_MM, tag="wt",
                                             name=f"wt{q}_{g}_{h2}")
                                nc.sync.dma_start(
                                    out=wt[:],
                                    in_=w_d.ap()[c, :, h2 * ksub : (h2 + 1) * ksub, :],
                                )
                                halves.append(wt)
                            wts.append(halves)
                    for m in range(MC):
                        ms = slice(m * MT, (m + 1) * MT)
                        ps = [
                            pp.tile([128, MT], mybir.dt.float32, name=f"ps{g}", tag=f"ps{g}")
                            for g in range(4)
                        ]
                        # g-outer/k-inner: chain g completes after only its
                        # own W tile + the combined chunks, and its activation
                        # overlaps the remaining chains
                        for g in range(4):
                            for k in range(KC):
                                nc.tensor.matmul(
                                    ps[g][:],
                                    lhsT=wts[g][k // ksub][:, k % ksub, :],
                                    rhs=cts[k][m][:],
                                    start=(k == 0),
                                    stop=(k == KC - 1),
                                )
                        # epilogue: chains finish in order f,i,cl,o; o's
                        # sigmoid + final mul are the only ops after the last
                        # matmul of the group. cp load emitted after the MMs so
                        # W tiles keep DMA queue priority.
                        cp_t = cpp.tile([128, MT], mybir.dt.float32, tag="cp")
                        nc.sync.dma_start(out=cp_t[:], in_=cp_d.ap()[:, q, ms])
                        c0 = q * 4
                        f_sb = gp.tile([128, MT], mybir.dt.float32, tag="f", name="f_sb")
                        i_sb = gp.tile([128, MT], mybir.dt.float32, tag="i", name="i_sb")
                        o_sb = gp.tile([128, MT], mybir.dt.float32, tag="o", name="o_sb")
                        cl_sb = gp.tile([128, MT], mybir.dt.float32, tag="cl", name="cl_sb")
                        nc.scalar.activation(f_sb[:], ps[0][:], _SIG, bias=bt_sb[:, c0 : c0 + 1])
                        nc.scalar.activation(i_sb[:], ps[1][:], _SIG, bias=bt_sb[:, c0 + 1 : c0 + 2])
                        nc.scalar.activation(cl_sb[:], ps[2][:], _TANH, bias=bt_sb[:, c0 + 3 : c0 + 4])
                        # C_t = f*C_prev + i*ctilda ; h_t = o*tanh(C_t)
                        t1 = ep.tile([128, MT], mybir.dt.float32, tag="t1", name="t1")
                        t2 = ep.tile([128, MT], mybir.dt.float32, tag="t2", name="t2")
                        c_out = ep.tile([128, MT], mybir.dt.float32, tag="c_out", name="c_out")
                        th = ep.tile([128, MT], mybir.dt.float32, tag="th", name="th")
                        h_out = ep.tile([128, MT], mybir.dt.float32, tag="h_out", name="h_out")
                        nc.vector.tensor_tensor(
                            t1[:], f_sb[:], cp_t[:], mybir.AluOpType.mult
                        )
                        nc.vector.tensor_tensor(
                            t2[:], i_sb[:], cl_sb[:], mybir.AluOpType.mult
                        )
                        nc.vector.tensor_tensor(
                            c_out[:], t1[:], t2[:], mybir.AluOpType.add
                        )
                        nc.scalar.activation(th[:], c_out[:], _TANH)
                        nc.sync.dma_start(out=ct_d.ap()[q, :, ms], in_=c_out[:])
                        last = q == QC - 1 and m == MC - 1
                        if last:
                            # split the final o->h chain so ACT/DVE/DMA overlap
                            # after the very last matmul
                            hw_ = MT // 2
                            for s in range(2):
                                sl = slice(s * hw_, (s + 1) * hw_)
                                osl = slice(m * MT + s * hw_, m * MT + (s + 1) * hw_)
                                nc.scalar.activation(
                                    o_sb[:, sl], ps[3][:, sl], _SIG,
                                    bias=bt_sb[:, c0 + 2 : c0 + 3],
                                )
                                nc.vector.tensor_tensor(
                                    h_out[:, sl], o_sb[:, sl], th[:, sl],
                                    mybir.AluOpType.mult,
                                )
                                nc.sync.dma_start(
                                    out=ht_d.ap()[q, :, osl], in_=h_out[:, sl]
                                )
                        else:
                            nc.scalar.activation(o_sb[:], ps[3][:], _SIG, bias=bt_sb[:, c0 + 2 : c0 + 3])
                            nc.vector.tensor_tensor(
                                h_out[:], o_sb[:], th[:], mybir.AluOpType.mult
                            )
                            nc.sync.dma_start(out=ht_d.ap()[q, :, ms], in_=h_out[:])
    nc.compile()
    return nc


def prep_inputs(x, h_prev, C_prev, W_f, b_f, W_i, b_i, W_C, b_C, W_o, b_o):
    """Shard + lay out host arrays for the device program. Returns in_maps."""
    f32 = np.float32
    x = np.ascontiguousarray(x, f32)
    h_prev = np.ascontiguousarray(h_prev, f32)
    C_prev = np.ascontiguousarray(C_prev, f32)

    # W tiles: w5[c, p, ko, n] = W_gate[ko*128+p, q*128+n], c = q*4+g
    # Build as [QC, 4, 128(p), KC, 128(n)] then reshape.
    w5 = np.empty((QC, 4, 128, KC, 128), f32)
    for g, Wg in enumerate((W_f, W_i, W_o, W_C)):
        Wg = np.ascontiguousarray(Wg, f32)
        # [K, H] -> [KC, 128(p), QC, 128(n)] -> (q, p, ko, n)
        wr = Wg.reshape(KC, 128, QC, 128)
        w5[:, g] = wr.transpose(2, 1, 0, 3)
    w5 = np.ascontiguousarray(w5.reshape(NCHUNKS, 128, KC, 128).astype(_NP_MM))

    bt = np.empty((QC, 4, 128), f32)
    for g, bg in enumerate((b_f, b_i, b_o, b_C)):
        bt[:, g] = np.asarray(bg, f32).reshape(QC, 128)
    bt = np.ascontiguousarray(bt.reshape(NCHUNKS, 128).T)  # [128, NCHUNKS]

    in_maps = []
    for c in range(N_CORES):
        rs = slice(c * BL, (c + 1) * BL)
        # combined^T: [128(p), KC, BL]; rows 0..I-1 = x^T, I..K-1 = h^T
        comb = np.empty((KC, 128, BL), f32)
        comb.reshape(K, BL)[:I] = x[rs].T
        comb.reshape(K, BL)[I:] = h_prev[rs].T
        comb = np.ascontiguousarray(comb.transpose(1, 0, 2).astype(_NP_MM))
        # C_prev^T: [128(p), QC, BL]
        cp = np.ascontiguousarray(
            C_prev[rs].T.reshape(QC, 128, BL).transpose(1, 0, 2)
        )
        in_maps.append({"comb": comb, "w": w5, "bt": bt, "cp": cp})
    return in_maps


def assemble_outputs(results):
    """Gather per-core [QC, 128, BL] outputs into full [B, H] h_t, C_t."""
    h_t = np.empty((B, H), np.float32)
    C_t = np.empty((B, H), np.float32)
    for c, r in enumerate(results):
        rs = slice(c * BL, (c + 1) * BL)
        # [QC, 128, BL] -> [BL, QC*128]
        h_t[rs] = r["ht"].reshape(H, BL).T
        C_t[rs] = r["ct"].reshape(H, BL).T
    return h_t, C_t


_NC_CACHE = {}


def kernel(**inputs):
    if "nc" not in _NC_CACHE:
        _NC_CACHE["nc"] = build_program(repeats=1)
    nc = _NC_CACHE["nc"]
    in_maps = prep_inputs(**inputs)
    res = run_bass_kernel_spmd(nc, in_maps, core_ids=list(range(N_CORES)))
    return assemble_outputs(res.results)



# revision 11
# speedup vs baseline: 1.3565x; 1.3565x over previous
"""Trainium2 Bass kernel for a fused LSTM cell.

Problem: B=8192, I=H=1024.
  gates = [x, h_prev] @ [W_f|W_i|W_o|W_C] + b      # [B, 4H]
  C_t = sigmoid(f)*C_prev + sigmoid(i)*tanh(c)
  h_t = sigmoid(o)*tanh(C_t)

Sharding: data-parallel over batch across 8 NeuronCores (1024 rows each),
weights replicated. No collectives needed.

Per-core device program (mixed bf16 / fp8 matmuls, rel err ~1.2e-2 vs
the 2e-2 gate):
  - f, C~, o gates in bf16 (full PE rate, 1 cycle/row); the i gate in
    fp8e4m3 with MatmulPerfMode.DoubleRow (2 K-chunks per instruction at
    0.5 cycles/row). The i gate is the only one whose fp8 quantization
    error survives the LSTM's damping structure within tolerance: C~'s
    tanh passes error straight through (3.9e-2, fails), f's error is
    amplified by C_prev (|C|<=5.6), o multiplies h directly; i's error is
    damped by sigmoid' <= 0.25 and |C~| <= 1 (measured 1.17e-2 on HW).
  - combined^T (K=2048 x M=1024) resident in SBUF twice (bf16 + fp8),
    K on partitions; W_i pre-scaled by 512 into e4m3 normal range and
    descaled via the ScalarE activation's scale operand.
  - W streamed as bf16 [128, 8, 128] tiles; hidden dim on PSUM partitions
    so the per-gate bias rides the ScalarE activation's per-partition
    bias operand: gate = act(psum * scale + b).
  - Loop q (8 H-chunks of 128) x m (2 batch-chunks of 512): 4 chains into
    4 PSUM banks (bufs=2 -> all 8 banks), ScalarE sigmoid/tanh eviction,
    VectorE elementwise for C_t / h_t; h/C staged per-q in [128, 1024]
    bf16 tiles so output DMA lines stay 2KB (1KB bf16 lines measurably
    degrade HBM efficiency under 8-core contention).

Key HW findings baked in: the 8 cores contend for shared HBM (~170 GB/s
per core sustained, not the single-core 360 GB/s) so fp32 weights leave
the PE idle; bf16/fp8 operand+IO traffic (20 MB/pass) hides fully under
compute. LD_WEIGHTS is fully overlapped by row streaming (deduping it
gained nothing). All host-side layout shuffles are numpy copies outside
the measured device execution.
"""

import ml_dtypes
import numpy as np

import concourse.bass as bass
import concourse.mybir as mybir
import concourse.tile as tile
from concourse import bacc
from concourse.bass_utils import run_bass_kernel_spmd

N_CORES = 8
B, I, H = 8192, 1024, 1024
K = I + H                      # 2048 contraction dim
BL = B // N_CORES              # 1024 batch rows per core
KC = K // 128                  # 16 K-chunks
QC = H // 128                  # 8 hidden chunks of 128
MC = 2                         # batch chunks of 512 per core
MT = BL // MC                  # 512
NCHUNKS = 3 * QC               # bf16 W N-chunks (f, C~, o); i gate is fp8
NBIAS = 4 * QC                 # bias chunks still cover all 4 gates
FP8_SCALE = 512.0              # W_i upscale into e4m3 normal range

# bf16 operands: same full PE rate as fp32r (1.0 cycles/row at free dim
# >= 256) but half the HBM traffic for the streamed W — the 8 cores
# contend for shared HBM bandwidth (~170 GB/s/core sustained, not the
# single-core 360 GB/s), and fp32 weights left the PE waiting on DMA.
# Measured rel err 2.5e-3 vs the 2e-2 gate.
_DT_MM = mybir.dt.bfloat16     # matmul operand dtype
_DT_IO = mybir.dt.bfloat16     # C_prev input + h_t/C_t output dtype
_NP_IO = ml_dtypes.bfloat16


def set_mm_dtype(name):
    """Switch matmul operand dtype ('fp32r' | 'bf16' | 'fp32'). Test-only."""
    global _DT_MM, _NP_MM
    _DT_MM = {"fp32r": mybir.dt.float32r, "bf16": mybir.dt.bfloat16,
              "fp32": mybir.dt.float32}[name]
    _NP_MM = ml_dtypes.bfloat16 if name == "bf16" else np.float32
    _NC_CACHE.clear()


_NP_MM = ml_dtypes.bfloat16

# bf16 W chunk order within a q: 0=f, 1=C~, 2=o (i is the fp8 gate).
# Chain order on PE: f, i(fp8), C~, o — o last so the final epilogue's
# critical path after the last matmul is just sigmoid(o)*tanh(C_t)
GATE_ORDER = (0, 1, 2)

_SIG = mybir.ActivationFunctionType.Sigmoid
_TANH = mybir.ActivationFunctionType.Tanh


def build_program(repeats: int = 1):
    """Build the per-core Bass program. `repeats` unrolls the whole body
    (same data) for slope-based HW timing in test harnesses."""
    nc = bacc.Bacc("TRN2", target_bir_lowering=False, debug=False)

    # Host-prepped layouts (see prep_inputs):
    #   comb: [128, KC, BL]   combined^T, partition-major contiguous
    #   w:    [NCHUNKS, 128, KC, 128]  W tiles, partition-major contiguous
    #   bt:   [128, NCHUNKS]  bias chunks
    #   cp:   [128, QC, BL]   C_prev^T
    comb_d = nc.dram_tensor("comb", [128, KC, BL], _DT_MM, kind="ExternalInput")
    w_d = nc.dram_tensor("w", [NCHUNKS, 128, KC, 128], _DT_MM, kind="ExternalInput")
    comb8_d = nc.dram_tensor("comb8", [128, KC, BL], mybir.dt.float8e4, kind="ExternalInput")
    w8_d = nc.dram_tensor("w8", [QC, 128, KC, 128], mybir.dt.float8e4, kind="ExternalInput")
    bt_d = nc.dram_tensor("bt", [128, NBIAS], mybir.dt.float32, kind="ExternalInput")
    cp_d = nc.dram_tensor("cp", [128, QC, BL], _DT_IO, kind="ExternalInput")
    ht_d = nc.dram_tensor("ht", [QC, 128, BL], _DT_IO, kind="ExternalOutput")
    ct_d = nc.dram_tensor("ct", [QC, 128, BL], _DT_IO, kind="ExternalOutput")

    with tile.TileContext(nc) as tc:
        with (
            tc.tile_pool(name="res", bufs=1) as res,
            tc.tile_pool(name="wp", bufs=16) as wp,
            tc.tile_pool(name="cpp", bufs=4) as cpp,
            tc.tile_pool(name="gp", bufs=2) as gp,
            tc.tile_pool(name="ep", bufs=2) as ep,
            tc.tile_pool(name="psum", bufs=2, space="PSUM") as pp,
        ):
            # q0's W tiles split into k-quarters so the first accumulation
            # chain starts after ~256KB instead of 1MB; combined^T split per
            # (K-chunk, m-half) for the same reason. GATE_ORDER puts the o
            # gate last so the final epilogue only waits on one activation.
            KSUB = 4
            # DMA emission ordered by first-use time: gate-0 W quarters, then
            # the m=0 combined chunks its chain consumes, then the remaining
            # gates' W, then the m=1 chunks.
            wts0 = [[] for _ in range(3)]
            cts = [[None] * MC for _ in range(KC)]

            def _load_wq0(g):
                for kq in range(KC // KSUB):
                    wt = wp.tile([128, KSUB, 128], _DT_MM, tag="wq0", name=f"wt0_{g}_{kq}")
                    nc.sync.dma_start(
                        out=wt[:], in_=w_d.ap()[g, :, kq * KSUB : (kq + 1) * KSUB, :]
                    )
                    wts0[g].append(wt)

            def _load_ct(k, m):
                ctk = res.tile([128, MT], _DT_MM, tag=f"ct{k}_{m}", name=f"ct{k}_{m}")
                nc.sync.dma_start(
                    out=ctk[:], in_=comb_d.ap()[:, k, m * MT : (m + 1) * MT]
                )
                cts[k][m] = ctk

            _load_wq0(0)
            for k in range(KC):
                _load_ct(k, 0)
            comb8_sb = res.tile([128, KC, BL], mybir.dt.float8e4, name="comb8_sb")
            nc.sync.dma_start(out=comb8_sb[:], in_=comb8_d.ap())
            _load_wq0(1)
            bt_sb = res.tile([128, NBIAS], mybir.dt.float32)
            nc.sync.dma_start(out=bt_sb[:], in_=bt_d.ap())
            _load_wq0(2)
            for k in range(KC):
                for m in range(1, MC):
                    _load_ct(k, m)

            for _ in range(repeats):
                for q in range(QC):
                    if q == 0 and wts0 is not None:
                        wts = wts0
                        wts0 = None
                        ksub = KSUB
                    else:
                        # halve W tiles so each chain starts after 512KB, not
                        # 1MB — closes PE wait-for-W gaps in the early phase
                        ksub = KC // 2
                        wts = []
                        for g in range(3):
                            c = q * 3 + g
                            halves = []
                            for h2 in range(2):
                                wt = wp.tile([128, ksub, 128], _DT_MM, tag="wt",
                                             name=f"wt{q}_{g}_{h2}")
                                nc.sync.dma_start(
                                    out=wt[:],
                                    in_=w_d.ap()[c, :, h2 * ksub : (h2 + 1) * ksub, :],
                                )
                                halves.append(wt)
                            wts.append(halves)
                    w8q = wp.tile([128, KC, 128], mybir.dt.float8e4, tag="w8",
                                  name=f"w8_{q}")
                    nc.sync.dma_start(out=w8q[:], in_=w8_d.ap()[q])
                    # cp loaded per q as one [128, BL] tile (2KB bf16 lines);
                    # emitted after the W loads so W keeps DMA priority.
                    cp_t = cpp.tile([128, BL], _DT_IO, tag="cp")
                    nc.sync.dma_start(out=cp_t[:], in_=cp_d.ap()[:, q, :])
                    # full-BL staging for h_t/C_t: one [128, BL] DMA per q
                    # keeps store lines at 2KB (bf16) — [128, MT] bf16 stores
                    # are 1KB lines, which measurably degrade HBM efficiency
                    # under 8-core contention.
                    c_st = ep.tile([128, BL], _DT_IO, tag="c_st", name="c_st")
                    h_st = ep.tile([128, BL], _DT_IO, tag="h_st", name="h_st")
                    for m in range(MC):
                        ms = slice(m * MT, (m + 1) * MT)
                        ps = [
                            pp.tile([128, MT], mybir.dt.float32, name=f"ps{g}", tag=f"ps{g}")
                            for g in range(4)
                        ]
                        # chain order: f, i(fp8 DoubleRow), C~, o. The i
                        # chain consumes 2 K-chunks per instruction at 0.5
                        # cycles/row, halving (or better) that gate's PE rows.
                        def _bf16_chain(g, p):
                            for k in range(KC):
                                nc.tensor.matmul(
                                    p[:],
                                    lhsT=wts[g][k // ksub][:, k % ksub, :],
                                    rhs=cts[k][m][:],
                                    start=(k == 0),
                                    stop=(k == KC - 1),
                                )
                        _bf16_chain(0, ps[0])                      # f
                        for kk in range(KC // 2):                  # i (fp8)
                            nc.tensor.matmul(
                                ps[1][:],
                                lhsT=w8q[:, 2 * kk : 2 * kk + 2, :],
                                rhs=comb8_sb[:, 2 * kk : 2 * kk + 2, ms],
                                start=(kk == 0),
                                stop=(kk == KC // 2 - 1),
                                perf_mode=mybir.MatmulPerfMode.DoubleRow,
                            )
                        _bf16_chain(1, ps[2])                      # C~
                        _bf16_chain(2, ps[3])                      # o
                        # epilogue: chains finish in order f,i,cl,o; o's
                        # sigmoid + final mul are the only ops after the last
                        # matmul of the group.
                        c0 = q * 4
                        f_sb = gp.tile([128, MT], mybir.dt.float32, tag="f", name="f_sb")
                        i_sb = gp.tile([128, MT], mybir.dt.float32, tag="i", name="i_sb")
                        o_sb = gp.tile([128, MT], mybir.dt.float32, tag="o", name="o_sb")
                        cl_sb = gp.tile([128, MT], mybir.dt.float32, tag="cl", name="cl_sb")
                        nc.scalar.activation(f_sb[:], ps[0][:], _SIG, bias=bt_sb[:, c0 : c0 + 1])
                        nc.scalar.activation(i_sb[:], ps[1][:], _SIG, bias=bt_sb[:, c0 + 1 : c0 + 2],
                                             scale=1.0 / FP8_SCALE)
                        nc.scalar.activation(cl_sb[:], ps[2][:], _TANH, bias=bt_sb[:, c0 + 3 : c0 + 4])
                        # C_t = f*C_prev + i*ctilda ; h_t = o*tanh(C_t)
                        t1 = ep.tile([128, MT], mybir.dt.float32, tag="t1", name="t1")
                        t2 = ep.tile([128, MT], mybir.dt.float32, tag="t2", name="t2")
                        th = ep.tile([128, MT], mybir.dt.float32, tag="th", name="th")
                        nc.vector.tensor_tensor(
                            t1[:], f_sb[:], cp_t[:, ms], mybir.AluOpType.mult
                        )
                        nc.vector.tensor_tensor(
                            t2[:], i_sb[:], cl_sb[:], mybir.AluOpType.mult
                        )
                        nc.vector.tensor_tensor(
                            c_st[:, ms], t1[:], t2[:], mybir.AluOpType.add
                        )
                        nc.scalar.activation(th[:], c_st[:, ms], _TANH)
                        nc.scalar.activation(o_sb[:], ps[3][:], _SIG, bias=bt_sb[:, c0 + 2 : c0 + 3])
                        nc.vector.tensor_tensor(
                            h_st[:, ms], o_sb[:], th[:], mybir.AluOpType.mult
                        )
                    nc.sync.dma_start(out=ct_d.ap()[q, :, :], in_=c_st[:])
                    nc.sync.dma_start(out=ht_d.ap()[q, :, :], in_=h_st[:])
    nc.compile()
    return nc


def prep_inputs(x, h_prev, C_prev, W_f, b_f, W_i, b_i, W_C, b_C, W_o, b_o):
    """Shard + lay out host arrays for the device program. Returns in_maps."""
    f32 = np.float32
    x = np.ascontiguousarray(x, f32)
    h_prev = np.ascontiguousarray(h_prev, f32)
    C_prev = np.ascontiguousarray(C_prev, f32)

    # bf16 W tiles: w5[c, p, ko, n] = W_gate[ko*128+p, q*128+n], c = q*3+g
    # over gates (f, C~, o); the i gate ships separately as fp8.
    w5 = np.empty((QC, 3, 128, KC, 128), f32)
    for g, Wg in enumerate((W_f, W_C, W_o)):
        Wg = np.ascontiguousarray(Wg, f32)
        # [K, H] -> [KC, 128(p), QC, 128(n)] -> (q, p, ko, n)
        wr = Wg.reshape(KC, 128, QC, 128)
        w5[:, g] = wr.transpose(2, 1, 0, 3)
    w5 = np.ascontiguousarray(w5.reshape(NCHUNKS, 128, KC, 128).astype(_NP_MM))

    e4m3 = ml_dtypes.float8_e4m3fn
    wr8 = (np.ascontiguousarray(W_i, f32) * FP8_SCALE).reshape(KC, 128, QC, 128)
    w8 = np.ascontiguousarray(wr8.transpose(2, 1, 0, 3).astype(e4m3))

    bt = np.empty((QC, 4, 128), f32)
    for g, bg in enumerate((b_f, b_i, b_o, b_C)):
        bt[:, g] = np.asarray(bg, f32).reshape(QC, 128)
    bt = np.ascontiguousarray(bt.reshape(NBIAS, 128).T)  # [128, NBIAS]

    in_maps = []
    for c in range(N_CORES):
        rs = slice(c * BL, (c + 1) * BL)
        # combined^T: [128(p), KC, BL]; rows 0..I-1 = x^T, I..K-1 = h^T
        comb = np.empty((KC, 128, BL), f32)
        comb.reshape(K, BL)[:I] = x[rs].T
        comb.reshape(K, BL)[I:] = h_prev[rs].T
        comb = comb.transpose(1, 0, 2)
        comb8 = np.ascontiguousarray(comb.astype(e4m3))
        comb = np.ascontiguousarray(comb.astype(_NP_MM))
        # C_prev^T: [128(p), QC, BL]
        cp = np.ascontiguousarray(
            C_prev[rs].T.reshape(QC, 128, BL).transpose(1, 0, 2).astype(_NP_IO)
        )
        in_maps.append({"comb": comb, "w": w5, "comb8": comb8, "w8": w8,
                        "bt": bt, "cp": cp})
    return in_maps


def assemble_outputs(results):
    """Gather per-core [QC, 128, BL] outputs into full [B, H] h_t, C_t."""
    h_t = np.empty((B, H), np.float32)
    C_t = np.empty((B, H), np.float32)
    for c, r in enumerate(results):
        rs = slice(c * BL, (c + 1) * BL)
        # [QC, 128, BL] -> [BL, QC*128]
        h_t[rs] = r["ht"].reshape(H, BL).T
        C_t[rs] = r["ct"].reshape(H, BL).T
    return h_t, C_t


_NC_CACHE = {}


def kernel(**inputs):
    if "nc" not in _NC_CACHE:
        _NC_CACHE["nc"] = build_program(repeats=1)
    nc = _NC_CACHE["nc"]
    in_maps = prep_inputs(**inputs)
    res = run_bass_kernel_spmd(nc, in_maps, core_ids=list(range(N_CORES)))
    return assemble_outputs(res.results)

